# revision 40
# baseline (speedup 1.0000x reference)
"""Multi-head attention (B=2, S=2048, D=1024, H=16) on 8 trn2 NeuronCores.

Sharding: core c -> batch b = c // 4, head group g = c % 4 (heads 4g..4g+3).
Each core computes, for its batch shard and 4 heads:
  QT/KT = (x W + b)^T in [d_local, seq] layout, V in [seq, d_local] layout,
  transposed scores S^T[k, q] = K Q^T (so softmax needs no transposes),
  exp via ACT (scale fused), PV matmul with an appended ones column which
  yields both the unnormalized context and the softmax row sums,
  normalization via a gpsimd partition-broadcast reciprocal multiply,
  and a partial output projection against a row shard of Wo.
Host sums the 4 partials per batch and adds the constant row bv @ Wo + bo
(softmax rows sum to one, so bv's contribution is a constant vector).

Emission order keeps the PE fed (HAM clock gate): Q/K dblk0 projections,
then head 0 attention with the V projection interleaved per key chunk,
then Q/K dblk1 projections (overlapping head 0's normalize chain), then
heads 1-3, then the output projection.
"""

import sys

sys.path.insert(0, "/opt/trn_rl_repo")

import numpy as np
import ml_dtypes

B = 2
S = 2048
D = 1024
H = 16
HD = 64
NCORES = 8
HPC = 4          # heads per core
DL = HPC * HD    # 256 local head dims per core
P = 128
KCH = S // P     # 16 key chunks
DCH = D // P     # 8 contraction chunks
TBLK = S // P    # 16 token blocks
SCALE = 1.0 / np.sqrt(HD)

# fp8e4m3 + DoubleRow PV halves PV matmul time but costs ~1.1e-2 extra
# relative error (fp8 noise on V / probabilities does not average away
# relative to the softmax-averaged context). Keep bf16 for accuracy.
USE_FP8_PV = False

_CACHE = {}


def _build():
    import concourse.bass as bass  # noqa: F401
    import concourse.mybir as mybir
    import concourse.tile as tile
    from concourse import bacc

    bf16 = mybir.dt.bfloat16
    f32 = mybir.dt.float32
    fp8 = mybir.dt.float8e4
    DR = mybir.MatmulPerfMode.DoubleRow
    Exp = mybir.ActivationFunctionType.Exp

    nc = bacc.Bacc("TRN2", target_bir_lowering=False, debug=False,
                   num_devices=NCORES)

    xT_d = nc.dram_tensor("xt", [D, S], bf16, kind="ExternalInput")
    wq_d = nc.dram_tensor("wq", [D, DL], bf16, kind="ExternalInput")
    wk_d = nc.dram_tensor("wk", [D, DL], bf16, kind="ExternalInput")
    wv_d = nc.dram_tensor("wv", [D, DL], bf16, kind="ExternalInput")
    wo_d = nc.dram_tensor("wo", [DL, D], bf16, kind="ExternalInput")
    bqk_d = nc.dram_tensor("bqk", [P, 4], f32, kind="ExternalInput")
    out_d = nc.dram_tensor("out", [S, D], f32, kind="ExternalOutput")

    with tile.TileContext(nc) as tc:
        with (
            tc.tile_pool(name="persist", bufs=1) as pp,
            tc.tile_pool(name="stream", bufs=3) as sp,
            tc.tile_pool(name="psum", bufs=2, space="PSUM") as ps,
        ):
            # ---- input loads (contiguous per-chunk DMAs, consumption order)
            bqk_s = pp.tile([P, 4], f32, tag="bqk", name="bqk_s")
            nc.sync.dma_start(bqk_s[:], bqk_d[:])
            wq_s = pp.tile([P, DCH, DL], bf16, tag="wq", name="wq_s")
            wk_s = pp.tile([P, DCH, DL], bf16, tag="wk", name="wk_s")
            wv_s = pp.tile([P, DCH, DL], bf16, tag="wv", name="wv_s")
            xts = [pp.tile([P, S], bf16, tag=f"xt{c}", name=f"xt{c}")
                   for c in range(DCH)]
            # spread loads over four engine DMA queues: x chunks alternate
            # sync/gpsimd, weights ride scalar/vector
            for c in range(DCH):
                nc.scalar.dma_start(
                    wq_s[:, c, :], wq_d[c * P:(c + 1) * P, :])
                xeng = nc.sync if c % 2 == 0 else nc.gpsimd
                if c == 0:
                    nc.sync.dma_start(xts[0][:, 0:512], xT_d[0:P, 0:512])
                    nc.sync.dma_start(xts[0][:, 512:S], xT_d[0:P, 512:S])
                else:
                    xeng.dma_start(xts[c][:], xT_d[c * P:(c + 1) * P, :])
                nc.scalar.dma_start(
                    wk_s[:, c, :], wk_d[c * P:(c + 1) * P, :])
            for c in range(DCH):
                nc.scalar.dma_start(
                    wv_s[:, c, :], wv_d[c * P:(c + 1) * P, :])
            wo_s = pp.tile([P, 2, D], bf16, tag="wo", name="wo_s")
            for dc in range(2):
                nc.gpsimd.dma_start(
                    wo_s[:, dc, :], wo_d[dc * P:(dc + 1) * P, :])

            qt = [None, None]
            kt = [None, None]

            def proj_alloc(which, dblk):
                nm = "qt" if which == 0 else "kt"
                t_sb = pp.tile([P, S], bf16, tag=f"{nm}{dblk}",
                               name=f"{nm}{dblk}")
                (qt if which == 0 else kt)[dblk] = t_sb

            def proj_half(which, dblk, half):
                """One projection half: 16 MMs + bias copy (short psum life)."""
                w_s, bcol = (wq_s, 0) if which == 0 else (wk_s, 2)
                t_sb = (qt if which == 0 else kt)[dblk]
                acc = ps.tile([P, 1024], f32, tag="work",
                              name=f"ps_p{which}{dblk}{half}")
                for kc in range(DCH):
                    for ns in range(2):
                        nc.tensor.matmul(
                            acc[:, ns * 512:(ns + 1) * 512],
                            w_s[:, kc, dblk * P:(dblk + 1) * P],
                            xts[kc][:, half * 1024 + ns * 512:
                                    half * 1024 + (ns + 1) * 512],
                            start=(kc == 0), stop=(kc == DCH - 1),
                        )
                nc.vector.tensor_scalar_add(
                    t_sb[:, half * 1024:(half + 1) * 1024],
                    acc[:],
                    bqk_s[:, bcol + dblk:bcol + dblk + 1],
                )

            # V is stored in k-chunk PAIRS: [128, 2, 4 heads * 68] where col
            # 68h+64 holds the softmax-sum ones column (65..67 pad keeps the
            # pair step 16B-aligned for DoubleRow).
            att_dt = fp8 if USE_FP8_PV else bf16
            vts = [None] * (TBLK // 2)

            def v_proj(tb):
                pr, j = tb // 2, tb % 2
                if j == 0:
                    vt = pp.tile([P, 2, HPC * 68], att_dt, tag=f"v{pr}",
                                 name=f"v{pr}")
                    v4 = vt.rearrange("p j (h e) -> p j h e", e=68)
                    nc.gpsimd.memset(v4[:, :, :, 64:65], 1.0)
                    vts[pr] = vt
                vt = vts[pr]
                v4 = vt.rearrange("p j (h e) -> p j h e", e=68)
                acc = ps.tile([P, 1024], f32, tag="work", name=f"ps_v{tb}")
                for kc in range(DCH):
                    nc.tensor.matmul(
                        acc[:, 0:DL],
                        xts[kc][:, tb * P:(tb + 1) * P],
                        wv_s[:, kc, :],
                        start=(kc == 0), stop=(kc == DCH - 1),
                    )
                nc.vector.tensor_copy(
                    v4[:, j, :, 0:64],
                    acc[:, 0:DL].rearrange("p (h e) -> p h e", e=64),
                )

            ctx_sb = [pp.tile([P, S], bf16, tag=f"ctx{dc}", name=f"ctx{dc}")
                      for dc in range(2)]
            ctx_ps_ref = [None]
            etps = [None] * (KCH // 2)

            def scores_chunk(h, kc):
                dblk = h // 2
                roff = 64 * (h % 2)
                pr, j = kc // 2, kc % 2
                if j == 0:
                    etps[pr] = sp.tile([P, 2, S], att_dt, tag="expt", bufs=4,
                                       name=f"expt{h}_{pr}")
                et = etps[pr]
                for half in range(2):
                    sc = ps.tile([P, 1024], f32, tag="work",
                                 name=f"ps_sc{h}_{kc}_{half}")
                    for ns in range(2):
                        nc.tensor.matmul(
                            sc[:, ns * 512:(ns + 1) * 512],
                            kt[dblk][roff:roff + 64, kc * P:(kc + 1) * P],
                            qt[dblk][roff:roff + 64,
                                     half * 1024 + ns * 512:
                                     half * 1024 + (ns + 1) * 512],
                            start=True, stop=True,
                        )
                    nc.scalar.activation(
                        et[:, j, half * 1024:(half + 1) * 1024], sc[:],
                        Exp, scale=float(SCALE),
                    )

            NPAIR = KCH // 2

            def pv_pair(h, pr):
                if pr == 0:
                    ctx_ps_ref[0] = ps.tile([P, S], f32, tag="ctx", bufs=1,
                                            name=f"ps_ctx{h}")
                ctx_ps = ctx_ps_ref[0]
                v4 = vts[pr].rearrange("p j (h e) -> p j h e", e=68)
                if USE_FP8_PV:
                    for ns in range(4):
                        nc.tensor.matmul(
                            ctx_ps[0:65, ns * 512:(ns + 1) * 512],
                            v4[:, :, h, 0:65],
                            etps[pr][:, :, ns * 512:(ns + 1) * 512],
                            start=(pr == 0), stop=(pr == NPAIR - 1),
                            perf_mode=DR,
                        )
                else:
                    for j in range(2):
                        for ns in range(4):
                            nc.tensor.matmul(
                                ctx_ps[0:65, ns * 512:(ns + 1) * 512],
                                v4[:, j, h, 0:65],
                                etps[pr][:, j, ns * 512:(ns + 1) * 512],
                                start=(pr == 0 and j == 0),
                                stop=(pr == NPAIR - 1 and j == 1),
                            )

            def normalize(h, part=0, nparts=2):
                """Normalize one 1/nparts slice of head h's context."""
                dblk = h // 2
                roff = 64 * (h % 2)
                w = S // nparts
                ctx_ps = ctx_ps_ref[0]
                hs = slice(part * w, (part + 1) * w)
                srow = sp.tile([1, w], f32, tag=f"srow{w}", bufs=2,
                               name=f"srow{h}_{part}")
                nc.vector.tensor_copy(srow[:], ctx_ps[64:65, hs])
                rec = sp.tile([1, w], f32, tag=f"rec{w}", bufs=2,
                              name=f"rec{h}_{part}")
                nc.vector.reciprocal_approx_fast(rec[:], srow[:])
                bc = sp.tile([64, w], f32, tag=f"bc{w}", bufs=2,
                             name=f"bc{h}_{part}")
                nc.gpsimd.partition_broadcast(bc[:], rec[:])
                nc.vector.tensor_mul(
                    ctx_sb[dblk][roff:roff + 64, hs],
                    ctx_ps[0:64, hs], bc[:])

            # ---- emission schedule ----
            # dense front: Q dblk0 then K dblk0
            for which in (0, 1):
                proj_alloc(which, 0)
                for half in range(2):
                    proj_half(which, 0, half)
            proj_alloc(0, 1)
            proj_alloc(1, 1)
            # fillers: h0 -> V proj per chunk; h1 -> the four dblk1
            # projection halves injected at chunks 0/4/8/12 (h1 reads dblk0)
            h1_fill = [(0, 1, 0), (0, 1, 1), (1, 1, 0), (1, 1, 1)]
            # PV emission plan (chunk-pair indices): the PV stream starts 5
            # scores-chunks in, so the previous head's normalize chain always
            # has PE runway.
            pv_plan = {5: [0], 6: [1], 7: [2], 9: [3], 11: [4], 13: [5],
                       15: [6]}
            outa = [None] * TBLK

            def outa_tb(tb):
                """dc0 (heads 0-1) part of the output projection, K=128."""
                acc = ps.tile([P, 1024], f32, tag="work", name=f"ps_oa{tb}")
                for ns in range(2):
                    nc.tensor.matmul(
                        acc[:, ns * 512:(ns + 1) * 512],
                        ctx_sb[0][:, tb * P:(tb + 1) * P],
                        wo_s[:, 0, ns * 512:(ns + 1) * 512],
                        start=True, stop=True,
                    )
                oa = pp.tile([P, D], bf16, tag=f"oa{tb}", name=f"oa{tb}")
                nc.vector.tensor_copy(oa[:], acc[:])
                outa[tb] = oa

            def outb2_tb(tb):
                """Head 2 part (K=64), accumulated onto the bf16 partial."""
                acc = ps.tile([P, 1024], f32, tag="work", name=f"ps_ob2{tb}")
                for ns in range(2):
                    nc.tensor.matmul(
                        acc[:, ns * 512:(ns + 1) * 512],
                        ctx_sb[1][0:64, tb * P:(tb + 1) * P],
                        wo_s[0:64, 1, ns * 512:(ns + 1) * 512],
                        start=True, stop=True,
                    )
                nc.vector.tensor_add(outa[tb][:], acc[:], outa[tb][:])

            for h in range(HPC):
                for kc in range(KCH):
                    if h == 0:
                        v_proj(kc)
                    elif h == 1 and kc % 4 == 0:
                        proj_half(*h1_fill[kc // 4])

                    scores_chunk(h, kc)
                    for pkc in pv_plan.get(kc, []):
                        pv_pair(h, pkc)
                pv_pair(h, NPAIR - 1)
                if h < HPC - 1:
                    normalize(h, 0, 2)
                    normalize(h, 1, 2)

            # ---- last-head normalize (quartered) + output projection ----
            def out_tb(tb):
                acc = ps.tile([P, 1024], f32, tag="work", name=f"ps_o{tb}")
                for dc in range(2):
                    for ns in range(2):
                        nc.tensor.matmul(
                            acc[:, ns * 512:(ns + 1) * 512],
                            ctx_sb[dc][:, tb * P:(tb + 1) * P],
                            wo_s[:, dc, ns * 512:(ns + 1) * 512],
                            start=(dc == 0), stop=(dc == 1),
                        )
                o_sb = sp.tile([P, D], f32, tag="osb", name=f"osb{tb}")
                nc.vector.tensor_copy(o_sb[:], acc[:])
                eng = nc.sync if tb % 2 == 0 else nc.gpsimd
                eng.dma_start(out_d[tb * P:(tb + 1) * P, :], o_sb[:])

            for qq in range(4):
                normalize(HPC - 1, qq, 4)
                for tb in range(4 * qq, 4 * qq + 4):
                    out_tb(tb)

    nc.compile()
    return nc


def _get_compiled():
    if "nc" not in _CACHE:
        _CACHE["nc"] = _build()
    return _CACHE["nc"]


def kernel(x, Wq, bq, Wk, bk, Wv, bv, Wo, bo):
    from concourse.bass_utils import run_bass_kernel_spmd

    nc = _get_compiled()
    x = np.asarray(x, dtype=np.float32)
    Wq, bq = np.asarray(Wq, np.float32), np.asarray(bq, np.float32)
    Wk, bk = np.asarray(Wk, np.float32), np.asarray(bk, np.float32)
    Wv, bv = np.asarray(Wv, np.float32), np.asarray(bv, np.float32)
    Wo, bo = np.asarray(Wo, np.float32), np.asarray(bo, np.float32)

    bf = ml_dtypes.bfloat16
    in_maps = []
    for c in range(NCORES):
        b, g = c // 4, c % 4
        cols = slice(g * DL, (g + 1) * DL)
        bq_l, bk_l = bq[cols], bk[cols]
        bqk = np.stack(
            [bq_l[0:P], bq_l[P:2 * P], bk_l[0:P], bk_l[P:2 * P]], axis=1)
        in_maps.append({
            "xt": np.ascontiguousarray(x[b].T).astype(bf),
            "wq": Wq[:, cols].astype(bf),
            "wk": Wk[:, cols].astype(bf),
            "wv": Wv[:, cols].astype(bf),
            "wo": Wo[cols, :].astype(bf),
            "bqk": np.ascontiguousarray(bqk, np.float32),
        })

    _CACHE["in_maps"] = in_maps
    res = run_bass_kernel_spmd(nc, in_maps, list(range(NCORES)))

    # constant row: bv @ Wo + bo (softmax rows sum to 1)
    const_row = bv.astype(np.float64) @ Wo.astype(np.float64) + bo
    out = np.zeros((B, S, D), np.float64)
    for c in range(NCORES):
        out[c // 4] += res.results[c]["out"].astype(np.float64)
    out += const_row
    return out.astype(np.float32)


# revision 42
# speedup vs baseline: 1.1797x; 1.1797x over previous
"""Multi-head attention (B=2, S=2048, D=1024, H=16) on 8 trn2 NeuronCores.

Sharding: core c -> batch b = c // 4, head group g = c % 4 (heads 4g..4g+3).
Each core computes, for its batch shard and 4 heads:
  QT/KT = (x W + b)^T in [d_local, seq] layout, V in [seq, d_local] layout,
  transposed scores S^T[k, q] = K Q^T (so softmax needs no transposes),
  exp via ACT (scale fused), PV matmul with an appended ones column which
  yields both the unnormalized context and the softmax row sums,
  normalization via a gpsimd partition-broadcast reciprocal multiply,
  and a partial output projection against a row shard of Wo.
Host sums the 4 partials per batch and adds the constant row bv @ Wo + bo
(softmax rows sum to one, so bv's contribution is a constant vector).

Emission order keeps the PE fed (HAM clock gate): Q/K dblk0 projections,
then head 0 attention with the V projection interleaved per key chunk,
then Q/K dblk1 projections (overlapping head 0's normalize chain), then
heads 1-3, then the output projection.
"""

import sys

sys.path.insert(0, "/opt/trn_rl_repo")

import numpy as np
import ml_dtypes

B = 2
S = 2048
D = 1024
H = 16
HD = 64
NCORES = 8
HPC = 4          # heads per core
DL = HPC * HD    # 256 local head dims per core
P = 128
KCH = S // P     # 16 key chunks
DCH = D // P     # 8 contraction chunks
TBLK = S // P    # 16 token blocks
SCALE = 1.0 / np.sqrt(HD)

# fp8e4m3 + DoubleRow PV halves PV matmul time but costs ~1.1e-2 extra
# relative error (fp8 noise on V / probabilities does not average away
# relative to the softmax-averaged context). Keep bf16 for accuracy.
USE_FP8_PV = False

_CACHE = {}


def _build():
    import concourse.bass as bass  # noqa: F401
    import concourse.mybir as mybir
    import concourse.tile as tile
    from concourse import bacc

    bf16 = mybir.dt.bfloat16
    f32 = mybir.dt.float32
    fp8 = mybir.dt.float8e4
    DR = mybir.MatmulPerfMode.DoubleRow
    Exp = mybir.ActivationFunctionType.Exp

    nc = bacc.Bacc("TRN2", target_bir_lowering=False, debug=False,
                   num_devices=NCORES)

    xT_d = nc.dram_tensor("xt", [D, S], bf16, kind="ExternalInput")
    wq_d = nc.dram_tensor("wq", [D, DL], bf16, kind="ExternalInput")
    wk_d = nc.dram_tensor("wk", [D, DL], bf16, kind="ExternalInput")
    wv_d = nc.dram_tensor("wv", [D, DL], bf16, kind="ExternalInput")
    wo_d = nc.dram_tensor("wo", [DL, D], bf16, kind="ExternalInput")
    bqk_d = nc.dram_tensor("bqk", [P, 4], f32, kind="ExternalInput")
    out_d = nc.dram_tensor("out", [S, D], f32, kind="ExternalOutput")

    with tile.TileContext(nc) as tc:
        with (
            tc.tile_pool(name="persist", bufs=1) as pp,
            tc.tile_pool(name="stream", bufs=3) as sp,
            tc.tile_pool(name="psum", bufs=2, space="PSUM") as ps,
        ):
            # ---- input loads (contiguous per-chunk DMAs, consumption order)
            bqk_s = pp.tile([P, 4], f32, tag="bqk", name="bqk_s")
            nc.sync.dma_start(bqk_s[:], bqk_d[:])
            wq_s = pp.tile([P, DCH, DL], bf16, tag="wq", name="wq_s")
            wk_s = pp.tile([P, DCH, DL], bf16, tag="wk", name="wk_s")
            wv_s = pp.tile([P, DCH, DL], bf16, tag="wv", name="wv_s")
            xts = [pp.tile([P, S], bf16, tag=f"xt{c}", name=f"xt{c}")
                   for c in range(DCH)]
            for c in range(DCH):
                nc.gpsimd.dma_start(
                    wq_s[:, c, :], wq_d[c * P:(c + 1) * P, :])
                if c == 0:
                    nc.sync.dma_start(xts[0][:, 0:512], xT_d[0:P, 0:512])
                    nc.sync.dma_start(xts[0][:, 512:S], xT_d[0:P, 512:S])
                else:
                    nc.sync.dma_start(xts[c][:], xT_d[c * P:(c + 1) * P, :])
                nc.gpsimd.dma_start(
                    wk_s[:, c, :], wk_d[c * P:(c + 1) * P, :])
            for c in range(DCH):
                nc.gpsimd.dma_start(
                    wv_s[:, c, :], wv_d[c * P:(c + 1) * P, :])
            wo_s = pp.tile([P, 2, D], bf16, tag="wo", name="wo_s")
            for dc in range(2):
                nc.gpsimd.dma_start(
                    wo_s[:, dc, :], wo_d[dc * P:(dc + 1) * P, :])

            qt = [None, None]
            kt = [None, None]

            def proj_alloc(which, dblk):
                nm = "qt" if which == 0 else "kt"
                t_sb = pp.tile([P, S], bf16, tag=f"{nm}{dblk}",
                               name=f"{nm}{dblk}")
                (qt if which == 0 else kt)[dblk] = t_sb

            def proj_half(which, dblk, half):
                """One projection half: 16 MMs + bias copy (short psum life)."""
                w_s, bcol = (wq_s, 0) if which == 0 else (wk_s, 2)
                t_sb = (qt if which == 0 else kt)[dblk]
                acc = ps.tile([P, 1024], f32, tag="work",
                              name=f"ps_p{which}{dblk}{half}")
                for kc in range(DCH):
                    for ns in range(2):
                        nc.tensor.matmul(
                            acc[:, ns * 512:(ns + 1) * 512],
                            w_s[:, kc, dblk * P:(dblk + 1) * P],
                            xts[kc][:, half * 1024 + ns * 512:
                                    half * 1024 + (ns + 1) * 512],
                            start=(kc == 0), stop=(kc == DCH - 1),
                        )
                nc.vector.tensor_scalar_add(
                    t_sb[:, half * 1024:(half + 1) * 1024],
                    acc[:],
                    bqk_s[:, bcol + dblk:bcol + dblk + 1],
                )

            # V is stored in k-chunk PAIRS: [128, 2, 4 heads * 68] where col
            # 68h+64 holds the softmax-sum ones column (65..67 pad keeps the
            # pair step 16B-aligned for DoubleRow).
            att_dt = fp8 if USE_FP8_PV else bf16
            vts = [None] * (TBLK // 2)

            def v_proj(tb):
                pr, j = tb // 2, tb % 2
                if j == 0:
                    vt = pp.tile([P, 2, HPC * 68], att_dt, tag=f"v{pr}",
                                 name=f"v{pr}")
                    v4 = vt.rearrange("p j (h e) -> p j h e", e=68)
                    nc.gpsimd.memset(v4[:, :, :, 64:65], 1.0)
                    vts[pr] = vt
                vt = vts[pr]
                v4 = vt.rearrange("p j (h e) -> p j h e", e=68)
                acc = ps.tile([P, 1024], f32, tag="work", name=f"ps_v{tb}")
                for kc in range(DCH):
                    nc.tensor.matmul(
                        acc[:, 0:DL],
                        xts[kc][:, tb * P:(tb + 1) * P],
                        wv_s[:, kc, :],
                        start=(kc == 0), stop=(kc == DCH - 1),
                    )
                nc.vector.tensor_copy(
                    v4[:, j, :, 0:64],
                    acc[:, 0:DL].rearrange("p (h e) -> p h e", e=64),
                )

            ctx_sb = [pp.tile([P, S], bf16, tag=f"ctx{dc}", name=f"ctx{dc}")
                      for dc in range(2)]
            ctx_ps_ref = [None]
            etps = [None] * (KCH // 2)

            def scores_chunk(h, kc):
                dblk = h // 2
                roff = 64 * (h % 2)
                pr, j = kc // 2, kc % 2
                if j == 0:
                    etps[pr] = sp.tile([P, 2, S], att_dt, tag="expt", bufs=4,
                                       name=f"expt{h}_{pr}")
                et = etps[pr]
                for half in range(2):
                    sc = ps.tile([P, 1024], f32, tag="work",
                                 name=f"ps_sc{h}_{kc}_{half}")
                    for ns in range(2):
                        nc.tensor.matmul(
                            sc[:, ns * 512:(ns + 1) * 512],
                            kt[dblk][roff:roff + 64, kc * P:(kc + 1) * P],
                            qt[dblk][roff:roff + 64,
                                     half * 1024 + ns * 512:
                                     half * 1024 + (ns + 1) * 512],
                            start=True, stop=True,
                        )
                    nc.scalar.activation(
                        et[:, j, half * 1024:(half + 1) * 1024], sc[:],
                        Exp, scale=float(SCALE),
                    )

            NPAIR = KCH // 2

            def pv_pair(h, pr):
                if pr == 0:
                    ctx_ps_ref[0] = ps.tile([P, S], f32, tag="ctx", bufs=1,
                                            name=f"ps_ctx{h}")
                ctx_ps = ctx_ps_ref[0]
                v4 = vts[pr].rearrange("p j (h e) -> p j h e", e=68)
                if USE_FP8_PV:
                    for ns in range(4):
                        nc.tensor.matmul(
                            ctx_ps[0:65, ns * 512:(ns + 1) * 512],
                            v4[:, :, h, 0:65],
                            etps[pr][:, :, ns * 512:(ns + 1) * 512],
                            start=(pr == 0), stop=(pr == NPAIR - 1),
                            perf_mode=DR,
                        )
                else:
                    for j in range(2):
                        for ns in range(4):
                            nc.tensor.matmul(
                                ctx_ps[0:65, ns * 512:(ns + 1) * 512],
                                v4[:, j, h, 0:65],
                                etps[pr][:, j, ns * 512:(ns + 1) * 512],
                                start=(pr == 0 and j == 0),
                                stop=(pr == NPAIR - 1 and j == 1),
                            )

            def normalize(h, part=0, nparts=2):
                """Normalize one 1/nparts slice of head h's context."""
                dblk = h // 2
                roff = 64 * (h % 2)
                w = S // nparts
                ctx_ps = ctx_ps_ref[0]
                hs = slice(part * w, (part + 1) * w)
                srow = sp.tile([1, w], f32, tag=f"srow{w}", bufs=2,
                               name=f"srow{h}_{part}")
                nc.vector.tensor_copy(srow[:], ctx_ps[64:65, hs])
                rec = sp.tile([1, w], f32, tag=f"rec{w}", bufs=2,
                              name=f"rec{h}_{part}")
                nc.vector.reciprocal_approx_fast(rec[:], srow[:])
                bc = sp.tile([64, w], f32, tag=f"bc{w}", bufs=2,
                             name=f"bc{h}_{part}")
                nc.gpsimd.partition_broadcast(bc[:], rec[:])
                nc.vector.tensor_mul(
                    ctx_sb[dblk][roff:roff + 64, hs],
                    ctx_ps[0:64, hs], bc[:])

            # ---- emission schedule ----
            # dense front: Q dblk0 then K dblk0
            for which in (0, 1):
                proj_alloc(which, 0)
                for half in range(2):
                    proj_half(which, 0, half)
            proj_alloc(0, 1)
            proj_alloc(1, 1)
            # fillers: h0 -> V proj per chunk; h1 -> the four dblk1
            # projection halves injected at chunks 0/4/8/12 (h1 reads dblk0)
            h1_fill = [(0, 1, 0), (0, 1, 1), (1, 1, 0), (1, 1, 1)]
            # PV emission plan (chunk-pair indices): the PV stream starts 5
            # scores-chunks in, so the previous head's normalize chain always
            # has PE runway.
            pv_plan = {5: [0], 6: [1], 7: [2], 9: [3], 11: [4], 13: [5],
                       15: [6]}
            for h in range(HPC):
                for kc in range(KCH):
                    if h == 0:
                        v_proj(kc)
                    elif h == 1 and kc % 4 == 0:
                        proj_half(*h1_fill[kc // 4])

                    scores_chunk(h, kc)
                    for pkc in pv_plan.get(kc, []):
                        pv_pair(h, pkc)
                pv_pair(h, NPAIR - 1)
                if h < HPC - 1:
                    normalize(h, 0, 2)
                    normalize(h, 1, 2)

            # ---- last-head normalize (quartered) + output projection ----
            def out_tb(tb):
                acc = ps.tile([P, 1024], f32, tag="work", name=f"ps_o{tb}")
                for dc in range(2):
                    for ns in range(2):
                        nc.tensor.matmul(
                            acc[:, ns * 512:(ns + 1) * 512],
                            ctx_sb[dc][:, tb * P:(tb + 1) * P],
                            wo_s[:, dc, ns * 512:(ns + 1) * 512],
                            start=(dc == 0), stop=(dc == 1),
                        )
                o_sb = sp.tile([P, D], f32, tag="osb", name=f"osb{tb}")
                nc.vector.tensor_copy(o_sb[:], acc[:])
                eng = nc.sync if tb % 2 == 0 else nc.gpsimd
                eng.dma_start(out_d[tb * P:(tb + 1) * P, :], o_sb[:])

            for qq in range(4):
                normalize(HPC - 1, qq, 4)
                for tb in range(4 * qq, 4 * qq + 4):
                    out_tb(tb)

    nc.compile()
    return nc


def _get_compiled():
    if "nc" not in _CACHE:
        _CACHE["nc"] = _build()
    return _CACHE["nc"]


def kernel(x, Wq, bq, Wk, bk, Wv, bv, Wo, bo):
    from concourse.bass_utils import run_bass_kernel_spmd

    nc = _get_compiled()
    x = np.asarray(x, dtype=np.float32)
    Wq, bq = np.asarray(Wq, np.float32), np.asarray(bq, np.float32)
    Wk, bk = np.asarray(Wk, np.float32), np.asarray(bk, np.float32)
    Wv, bv = np.asarray(Wv, np.float32), np.asarray(bv, np.float32)
    Wo, bo = np.asarray(Wo, np.float32), np.asarray(bo, np.float32)

    bf = ml_dtypes.bfloat16
    in_maps = []
    for c in range(NCORES):
        b, g = c // 4, c % 4
        cols = slice(g * DL, (g + 1) * DL)
        bq_l, bk_l = bq[cols], bk[cols]
        bqk = np.stack(
            [bq_l[0:P], bq_l[P:2 * P], bk_l[0:P], bk_l[P:2 * P]], axis=1)
        in_maps.append({
            "xt": np.ascontiguousarray(x[b].T).astype(bf),
            "wq": Wq[:, cols].astype(bf),
            "wk": Wk[:, cols].astype(bf),
            "wv": Wv[:, cols].astype(bf),
            "wo": Wo[cols, :].astype(bf),
            "bqk": np.ascontiguousarray(bqk, np.float32),
        })

    _CACHE["in_maps"] = in_maps
    res = run_bass_kernel_spmd(nc, in_maps, list(range(NCORES)))

    # constant row: bv @ Wo + bo (softmax rows sum to 1)
    const_row = bv.astype(np.float64) @ Wo.astype(np.float64) + bo
    out = np.zeros((B, S, D), np.float64)
    for c in range(NCORES):
        out[c // 4] += res.results[c]["out"].astype(np.float64)
    out += const_row
    return out.astype(np.float32)


# revision 46
# speedup vs baseline: 1.1883x; 1.0073x over previous
"""Multi-head attention (B=2, S=2048, D=1024, H=16) on 8 trn2 NeuronCores.

Sharding: core c -> batch b = c // 4, head group g = c % 4 (heads 4g..4g+3).
Each core computes, for its batch shard and 4 heads:
  QT/KT = (x W + b)^T in [d_local, seq] layout, V in [seq, d_local] layout,
  transposed scores S^T[k, q] = K Q^T (so softmax needs no transposes),
  exp via ACT (scale fused), PV matmul with an appended ones column which
  yields both the unnormalized context and the softmax row sums,
  normalization via a gpsimd partition-broadcast reciprocal multiply,
  and a partial output projection against a row shard of Wo.
Host sums the 4 partials per batch and adds the constant row bv @ Wo + bo
(softmax rows sum to one, so bv's contribution is a constant vector).

Emission order keeps the PE fed (HAM clock gate): Q/K dblk0 projections,
then head 0 attention with the V projection interleaved per key chunk,
then Q/K dblk1 projections (overlapping head 0's normalize chain), then
heads 1-3, then the output projection.
"""

import sys

sys.path.insert(0, "/opt/trn_rl_repo")

import numpy as np
import ml_dtypes

B = 2
S = 2048
D = 1024
H = 16
HD = 64
NCORES = 8
HPC = 4          # heads per core
DL = HPC * HD    # 256 local head dims per core
P = 128
KCH = S // P     # 16 key chunks
DCH = D // P     # 8 contraction chunks
TBLK = S // P    # 16 token blocks
SCALE = 1.0 / np.sqrt(HD)

# fp8e4m3 + DoubleRow PV halves PV matmul time but costs ~1.1e-2 extra
# relative error (fp8 noise on V / probabilities does not average away
# relative to the softmax-averaged context). Keep bf16 for accuracy.
USE_FP8_PV = False

_CACHE = {}


def _build():
    import concourse.bass as bass  # noqa: F401
    import concourse.mybir as mybir
    import concourse.tile as tile
    from concourse import bacc

    bf16 = mybir.dt.bfloat16
    f32 = mybir.dt.float32
    fp8 = mybir.dt.float8e4
    DR = mybir.MatmulPerfMode.DoubleRow
    Exp = mybir.ActivationFunctionType.Exp

    nc = bacc.Bacc("TRN2", target_bir_lowering=False, debug=False,
                   num_devices=NCORES)

    xT_d = nc.dram_tensor("xt", [D, S], bf16, kind="ExternalInput")
    wq_d = nc.dram_tensor("wq", [D, DL], bf16, kind="ExternalInput")
    wk_d = nc.dram_tensor("wk", [D, DL], bf16, kind="ExternalInput")
    wv_d = nc.dram_tensor("wv", [D, DL], bf16, kind="ExternalInput")
    wo_d = nc.dram_tensor("wo", [DL, D], bf16, kind="ExternalInput")
    bqk_d = nc.dram_tensor("bqk", [P, 4], f32, kind="ExternalInput")
    out_d = nc.dram_tensor("out", [S, D], bf16, kind="ExternalOutput")

    with tile.TileContext(nc) as tc:
        with (
            tc.tile_pool(name="persist", bufs=1) as pp,
            tc.tile_pool(name="stream", bufs=3) as sp,
            tc.tile_pool(name="psum", bufs=2, space="PSUM") as ps,
        ):
            # ---- input loads (contiguous per-chunk DMAs, consumption order)
            bqk_s = pp.tile([P, 4], f32, tag="bqk", name="bqk_s")
            nc.sync.dma_start(bqk_s[:], bqk_d[:])
            wq_s = pp.tile([P, DCH, DL], bf16, tag="wq", name="wq_s")
            wk_s = pp.tile([P, DCH, DL], bf16, tag="wk", name="wk_s")
            wv_s = pp.tile([P, DCH, DL], bf16, tag="wv", name="wv_s")
            xts = [pp.tile([P, S], bf16, tag=f"xt{c}", name=f"xt{c}")
                   for c in range(DCH)]
            for c in range(DCH):
                nc.gpsimd.dma_start(
                    wq_s[:, c, :], wq_d[c * P:(c + 1) * P, :])
                if c == 0:
                    nc.sync.dma_start(xts[0][:, 0:512], xT_d[0:P, 0:512])
                    nc.sync.dma_start(xts[0][:, 512:S], xT_d[0:P, 512:S])
                else:
                    nc.sync.dma_start(xts[c][:], xT_d[c * P:(c + 1) * P, :])
                nc.gpsimd.dma_start(
                    wk_s[:, c, :], wk_d[c * P:(c + 1) * P, :])
            for c in range(DCH):
                nc.gpsimd.dma_start(
                    wv_s[:, c, :], wv_d[c * P:(c + 1) * P, :])
            wo_s = pp.tile([P, 2, D], bf16, tag="wo", name="wo_s")
            for dc in range(2):
                nc.gpsimd.dma_start(
                    wo_s[:, dc, :], wo_d[dc * P:(dc + 1) * P, :])

            qt = [None, None]
            kt = [None, None]

            def proj_alloc(which, dblk):
                nm = "qt" if which == 0 else "kt"
                t_sb = pp.tile([P, S], bf16, tag=f"{nm}{dblk}",
                               name=f"{nm}{dblk}")
                (qt if which == 0 else kt)[dblk] = t_sb

            def proj_half(which, dblk, half):
                """One projection half: 16 MMs + bias copy (short psum life)."""
                w_s, bcol = (wq_s, 0) if which == 0 else (wk_s, 2)
                t_sb = (qt if which == 0 else kt)[dblk]
                acc = ps.tile([P, 1024], f32, tag="work",
                              name=f"ps_p{which}{dblk}{half}")
                for kc in range(DCH):
                    for ns in range(2):
                        nc.tensor.matmul(
                            acc[:, ns * 512:(ns + 1) * 512],
                            w_s[:, kc, dblk * P:(dblk + 1) * P],
                            xts[kc][:, half * 1024 + ns * 512:
                                    half * 1024 + (ns + 1) * 512],
                            start=(kc == 0), stop=(kc == DCH - 1),
                        )
                nc.vector.tensor_scalar_add(
                    t_sb[:, half * 1024:(half + 1) * 1024],
                    acc[:],
                    bqk_s[:, bcol + dblk:bcol + dblk + 1],
                )

            # V is stored in k-chunk PAIRS: [128, 2, 4 heads * 68] where col
            # 68h+64 holds the softmax-sum ones column (65..67 pad keeps the
            # pair step 16B-aligned for DoubleRow).
            att_dt = fp8 if USE_FP8_PV else bf16
            vts = [None] * (TBLK // 2)

            def v_proj(tb):
                pr, j = tb // 2, tb % 2
                if j == 0:
                    vt = pp.tile([P, 2, HPC * 68], att_dt, tag=f"v{pr}",
                                 name=f"v{pr}")
                    v4 = vt.rearrange("p j (h e) -> p j h e", e=68)
                    nc.gpsimd.memset(v4[:, :, :, 64:65], 1.0)
                    vts[pr] = vt
                vt = vts[pr]
                v4 = vt.rearrange("p j (h e) -> p j h e", e=68)
                acc = ps.tile([P, 1024], f32, tag="work", name=f"ps_v{tb}")
                for kc in range(DCH):
                    nc.tensor.matmul(
                        acc[:, 0:DL],
                        xts[kc][:, tb * P:(tb + 1) * P],
                        wv_s[:, kc, :],
                        start=(kc == 0), stop=(kc == DCH - 1),
                    )
                nc.vector.tensor_copy(
                    v4[:, j, :, 0:64],
                    acc[:, 0:DL].rearrange("p (h e) -> p h e", e=64),
                )

            ctx_sb = [pp.tile([P, S], bf16, tag=f"ctx{dc}", name=f"ctx{dc}")
                      for dc in range(2)]
            ctx_ps_ref = [None]
            etps = [None] * (KCH // 2)

            def scores_chunk(h, kc):
                dblk = h // 2
                roff = 64 * (h % 2)
                pr, j = kc // 2, kc % 2
                if j == 0:
                    etps[pr] = sp.tile([P, 2, S], att_dt, tag="expt", bufs=4,
                                       name=f"expt{h}_{pr}")
                et = etps[pr]
                for half in range(2):
                    sc = ps.tile([P, 1024], f32, tag="work",
                                 name=f"ps_sc{h}_{kc}_{half}")
                    for ns in range(2):
                        nc.tensor.matmul(
                            sc[:, ns * 512:(ns + 1) * 512],
                            kt[dblk][roff:roff + 64, kc * P:(kc + 1) * P],
                            qt[dblk][roff:roff + 64,
                                     half * 1024 + ns * 512:
                                     half * 1024 + (ns + 1) * 512],
                            start=True, stop=True,
                        )
                    nc.scalar.activation(
                        et[:, j, half * 1024:(half + 1) * 1024], sc[:],
                        Exp, scale=float(SCALE),
                    )

            NPAIR = KCH // 2

            def pv_pair(h, pr):
                if pr == 0:
                    ctx_ps_ref[0] = ps.tile([P, S], f32, tag="ctx", bufs=1,
                                            name=f"ps_ctx{h}")
                ctx_ps = ctx_ps_ref[0]
                v4 = vts[pr].rearrange("p j (h e) -> p j h e", e=68)
                if USE_FP8_PV:
                    for ns in range(4):
                        nc.tensor.matmul(
                            ctx_ps[0:65, ns * 512:(ns + 1) * 512],
                            v4[:, :, h, 0:65],
                            etps[pr][:, :, ns * 512:(ns + 1) * 512],
                            start=(pr == 0), stop=(pr == NPAIR - 1),
                            perf_mode=DR,
                        )
                else:
                    for j in range(2):
                        for ns in range(4):
                            nc.tensor.matmul(
                                ctx_ps[0:65, ns * 512:(ns + 1) * 512],
                                v4[:, j, h, 0:65],
                                etps[pr][:, j, ns * 512:(ns + 1) * 512],
                                start=(pr == 0 and j == 0),
                                stop=(pr == NPAIR - 1 and j == 1),
                            )

            def normalize(h, part=0, nparts=2):
                """Normalize one 1/nparts slice of head h's context."""
                dblk = h // 2
                roff = 64 * (h % 2)
                w = S // nparts
                ctx_ps = ctx_ps_ref[0]
                hs = slice(part * w, (part + 1) * w)
                srow = sp.tile([1, w], f32, tag=f"srow{w}", bufs=2,
                               name=f"srow{h}_{part}")
                nc.vector.tensor_copy(srow[:], ctx_ps[64:65, hs])
                rec = sp.tile([1, w], f32, tag=f"rec{w}", bufs=2,
                              name=f"rec{h}_{part}")
                nc.vector.reciprocal_approx_fast(rec[:], srow[:])
                bc = sp.tile([64, w], f32, tag=f"bc{w}", bufs=2,
                             name=f"bc{h}_{part}")
                nc.gpsimd.partition_broadcast(bc[:], rec[:])
                nc.vector.tensor_mul(
                    ctx_sb[dblk][roff:roff + 64, hs],
                    ctx_ps[0:64, hs], bc[:])

            # ---- emission schedule ----
            # dense front: Q dblk0 then K dblk0
            for which in (0, 1):
                proj_alloc(which, 0)
                for half in range(2):
                    proj_half(which, 0, half)
            proj_alloc(0, 1)
            proj_alloc(1, 1)
            # fillers: h0 -> V proj per chunk; h1 -> the four dblk1
            # projection halves injected at chunks 0/4/8/12 (h1 reads dblk0)
            h1_fill = [(0, 1, 0), (0, 1, 1), (1, 1, 0), (1, 1, 1)]
            # PV emission plan (chunk-pair indices): the PV stream starts 5
            # scores-chunks in, so the previous head's normalize chain always
            # has PE runway.
            pv_plan = {5: [0], 6: [1], 7: [2], 9: [3], 11: [4], 13: [5],
                       15: [6]}
            for h in range(HPC):
                for kc in range(KCH):
                    if h == 0:
                        v_proj(kc)
                    elif h == 1 and kc % 4 == 0:
                        proj_half(*h1_fill[kc // 4])

                    scores_chunk(h, kc)
                    for pkc in pv_plan.get(kc, []):
                        pv_pair(h, pkc)
                pv_pair(h, NPAIR - 1)
                if h < HPC - 1:
                    normalize(h, 0, 2)
                    normalize(h, 1, 2)

            # ---- last-head normalize (quartered) + output projection ----
            def out_tb(tb):
                acc = ps.tile([P, 1024], f32, tag="work", name=f"ps_o{tb}")
                for dc in range(2):
                    for ns in range(2):
                        nc.tensor.matmul(
                            acc[:, ns * 512:(ns + 1) * 512],
                            ctx_sb[dc][:, tb * P:(tb + 1) * P],
                            wo_s[:, dc, ns * 512:(ns + 1) * 512],
                            start=(dc == 0), stop=(dc == 1),
                        )
                o_sb = sp.tile([P, D], bf16, tag="osb", name=f"osb{tb}")
                nc.vector.tensor_copy(o_sb[:], acc[:])
                eng = nc.sync if tb % 2 == 0 else nc.gpsimd
                eng.dma_start(out_d[tb * P:(tb + 1) * P, :], o_sb[:])

            for qq in range(4):
                normalize(HPC - 1, qq, 4)
                for tb in range(4 * qq, 4 * qq + 4):
                    out_tb(tb)

    nc.compile()
    return nc


def _get_compiled():
    if "nc" not in _CACHE:
        _CACHE["nc"] = _build()
    return _CACHE["nc"]


def kernel(x, Wq, bq, Wk, bk, Wv, bv, Wo, bo):
    from concourse.bass_utils import run_bass_kernel_spmd

    nc = _get_compiled()
    x = np.asarray(x, dtype=np.float32)
    Wq, bq = np.asarray(Wq, np.float32), np.asarray(bq, np.float32)
    Wk, bk = np.asarray(Wk, np.float32), np.asarray(bk, np.float32)
    Wv, bv = np.asarray(Wv, np.float32), np.asarray(bv, np.float32)
    Wo, bo = np.asarray(Wo, np.float32), np.asarray(bo, np.float32)

    bf = ml_dtypes.bfloat16
    in_maps = []
    for c in range(NCORES):
        b, g = c // 4, c % 4
        cols = slice(g * DL, (g + 1) * DL)
        bq_l, bk_l = bq[cols], bk[cols]
        bqk = np.stack(
            [bq_l[0:P], bq_l[P:2 * P], bk_l[0:P], bk_l[P:2 * P]], axis=1)
        in_maps.append({
            "xt": np.ascontiguousarray(x[b].T).astype(bf),
            "wq": Wq[:, cols].astype(bf),
            "wk": Wk[:, cols].astype(bf),
            "wv": Wv[:, cols].astype(bf),
            "wo": Wo[cols, :].astype(bf),
            "bqk": np.ascontiguousarray(bqk, np.float32),
        })

    _CACHE["in_maps"] = in_maps
    res = run_bass_kernel_spmd(nc, in_maps, list(range(NCORES)))

    # constant row: bv @ Wo + bo (softmax rows sum to 1)
    const_row = bv.astype(np.float64) @ Wo.astype(np.float64) + bo
    out = np.zeros((B, S, D), np.float64)
    for c in range(NCORES):
        out[c // 4] += res.results[c]["out"].astype(np.float64)
    out += const_row
    return out.astype(np.float32)
